# revision 3
# baseline (speedup 1.0000x reference)
"""KNO2d kernel for 8 NeuronCores (batch-sharded SPMD).

The 6-iteration Koopman loop x = x + irfft2(Z(M(rfft2(x)))) is linear in
mode space, so it collapses into a single per-mode linear map precomputed
on host, including the exact rfft/irfft Hermitian-symmetrization at ky=0
(mirror-pair coupling + one leak mode at kx=12). The FFTs touch only a
24x12 mode corner, so they are computed as real DFT matmuls (cos/sin
factors) instead of full FFTs. Device stage 1: enc conv + tanh + forward
partial DFT. Host: per-mode transform on the tiny [32,32,24,12] mode
tensor (float64). Device stage 2: inverse partial DFT + w0/dec convs.
x0 stays device-resident between the stages.
"""
import os
import numpy as np

B, CIN, COUT, OP = 32, 6, 4, 32
MX, MY, DEC = 12, 12, 6
H, W = 192, 192
NDEV = 8
BS = B // NDEV  # 4 samples per core


def _precompute(km_r, km_i):
    K = km_r.astype(np.float64) + 1j * km_i.astype(np.float64)  # [t,f,x,y]
    I = np.eye(OP)
    A6 = np.zeros((MX, MY, OP, OP), dtype=np.complex128)
    for x in range(MX):
        for y in range(MY):
            A6[x, y] = np.linalg.matrix_power(I + K[:, :, x, y], DEC) - I
    K00 = K[:, :, 0, 0]
    Kr, Ki = K00.real, K00.imag
    S = sum(np.linalg.matrix_power(I + Kr, j) for j in range(DEC))
    mats = dict(
        A6=A6,
        A0=np.linalg.matrix_power(I + Kr, DEC) - I,
        B0=-(Ki @ S),
        Pk=np.stack([np.linalg.matrix_power(
            I + (K[:, :, k, 0] + np.conj(K[:, :, MX - k, 0])) / 2, DEC) - I
            for k in range(1, MX)]),
        R=np.linalg.matrix_power(I + K00 / 2, DEC) - I,
    )
    ky = np.arange(MY)
    w = np.arange(W)
    h = np.arange(H)
    kx24 = np.concatenate([np.arange(MX), np.arange(H - MX, H)])
    kx25 = np.concatenate([np.arange(MX), [MX], np.arange(H - MX, H)])
    s = np.full(MY, 2.0 / (H * W)); s[0] = 1.0 / (H * W)
    f32 = lambda a: np.ascontiguousarray(a, dtype=np.float32)
    dft = dict(
        Cw=f32(np.cos(2 * np.pi * np.outer(w, ky) / W)),
        Sw=f32(np.sin(2 * np.pi * np.outer(w, ky) / W)),
        Ch24=f32(np.cos(2 * np.pi * np.outer(h, kx24) / H)),
        Sh24=f32(np.sin(2 * np.pi * np.outer(h, kx24) / H)),
        Che=f32(np.cos(2 * np.pi * np.outer(kx25, h) / H)),
        She=f32(np.sin(2 * np.pi * np.outer(kx25, h) / H)),
        A2=f32(s[:, None] * np.cos(2 * np.pi * np.outer(ky, w) / W)),
        B2=f32(s[:, None] * np.sin(2 * np.pi * np.outer(ky, w) / W)),
    )
    return mats, dft


def _mode_transform(G_lo, G_hi, m):
    """G_lo/G_hi: [N,OP,12,12] complex128 -> DF [N,OP,25,12] complex128."""
    n = G_lo.shape[0]
    DG_lo = np.zeros_like(G_lo)
    DG_hi = np.zeros_like(G_hi)
    A6 = m['A6']
    for y in range(1, MY):
        for x in range(MX):
            DG_lo[:, :, x, y] = G_lo[:, :, x, y] @ A6[x, y]
            DG_hi[:, :, x, y] = G_hi[:, :, x, y] @ A6[x, y]
    ur, ui = G_lo[:, :, 0, 0].real, G_lo[:, :, 0, 0].imag
    DG_lo[:, :, 0, 0] = ur @ m['A0'] + ui @ m['B0']
    for k in range(1, MX):
        sv = G_lo[:, :, k, 0] + np.conj(G_hi[:, :, MX - k, 0])
        D = (sv @ m['Pk'][k - 1]) / 2
        DG_lo[:, :, k, 0] = D
        DG_hi[:, :, MX - k, 0] = np.conj(D)
    u = G_hi[:, :, 0, 0]
    Dh0 = u @ m['R']
    DG_hi[:, :, 0, 0] = Dh0
    DF = np.zeros((n, OP, 2 * MX + 1, MY), dtype=np.complex128)
    DF[:, :, :MX] = DG_lo
    DF[:, :, MX, 0] = np.conj(Dh0)  # leak at kx=12
    DF[:, :, MX + 1:] = DG_hi
    return DF


def _stage1(x, enc_w, enc_b, Cw, Sw, Ch24, Sh24):
    import jax.numpy as jnp
    x0 = jnp.tanh(jnp.einsum('bchw,oc->bohw', x, enc_w)
                  + enc_b[None, :, None, None])
    Tr = jnp.einsum('bthw,wy->bthy', x0, Cw)
    Ti = -jnp.einsum('bthw,wy->bthy', x0, Sw)
    Gr = (jnp.einsum('bthy,hx->btxy', Tr, Ch24)
          + jnp.einsum('bthy,hx->btxy', Ti, Sh24))
    Gi = (jnp.einsum('bthy,hx->btxy', Ti, Ch24)
          - jnp.einsum('bthy,hx->btxy', Tr, Sh24))
    return x0, Gr, Gi


def _stage2(x0, DFr, DFi, w0_w, w0_b, dec_w, dec_b, Che, She, A2, B2):
    import jax.numpy as jnp
    Er = (jnp.einsum('bfxy,xh->bfhy', DFr, Che)
          - jnp.einsum('bfxy,xh->bfhy', DFi, She))
    Ei = (jnp.einsum('bfxy,xh->bfhy', DFr, She)
          + jnp.einsum('bfxy,xh->bfhy', DFi, Che))
    d = (jnp.einsum('bfhy,yw->bfhw', Er, A2)
         - jnp.einsum('bfhy,yw->bfhw', Ei, B2))
    x_raw = (jnp.einsum('bchw,oc->bohw', x0, w0_w)
             + w0_b[None, :, None, None] + x0 + d)
    out = (jnp.einsum('bchw,oc->bohw', jnp.tanh(x_raw), dec_w)
           + dec_b[None, :, None, None])
    return out, x_raw.reshape(BS, OP, H * W)


_CACHE = {}


def kernel(x, enc_w, enc_b, dec_w, dec_b, w0_w, w0_b, km_r, km_i):
    os.environ.setdefault('NEURON_CC_FLAGS', '--auto-cast=none')
    import jax
    mats, dft = _precompute(np.asarray(km_r), np.asarray(km_i))
    if 'f1' not in _CACHE:
        devs = jax.devices()[:NDEV]
        _CACHE['f1'] = jax.pmap(_stage1, in_axes=(0,) + (None,) * 6,
                                devices=devs)
        _CACHE['f2'] = jax.pmap(_stage2, in_axes=(0, 0, 0) + (None,) * 8,
                                devices=devs)
    xs = np.ascontiguousarray(np.asarray(x, np.float32).reshape(
        NDEV, BS, CIN, H, W))
    f32 = lambda a: np.asarray(a, np.float32)
    x0, Gr, Gi = _CACHE['f1'](xs, f32(enc_w), f32(enc_b), dft['Cw'],
                              dft['Sw'], dft['Ch24'], dft['Sh24'])
    G = (np.asarray(Gr, np.float64)
         + 1j * np.asarray(Gi, np.float64)).reshape(B, OP, 2 * MX, MY)
    DF = _mode_transform(G[:, :, :MX], G[:, :, MX:], mats)
    DFr = np.ascontiguousarray(
        DF.real.reshape(NDEV, BS, OP, 2 * MX + 1, MY), np.float32)
    DFi = np.ascontiguousarray(
        DF.imag.reshape(NDEV, BS, OP, 2 * MX + 1, MY), np.float32)
    out, x_raw = _CACHE['f2'](x0, DFr, DFi, f32(w0_w), f32(w0_b),
                              f32(dec_w), f32(dec_b), dft['Che'],
                              dft['She'], dft['A2'], dft['B2'])
    out = np.asarray(out).reshape(B, COUT, H, W).astype(np.float32)
    x_raw = np.asarray(x_raw).reshape(B, OP, H * W).astype(np.float32)
    return out, x_raw


# revision 4
# speedup vs baseline: 9.7216x; 9.7216x over previous
"""KNO2d kernel for 8 NeuronCores (batch-sharded SPMD).

The 6-iteration Koopman loop x = x + irfft2(Z(M(rfft2(x)))) is linear in
mode space, so it collapses into a single per-mode linear map precomputed
on host, including the exact rfft/irfft Hermitian-symmetrization at ky=0
(mirror-pair coupling + one leak mode at kx=12). The FFTs touch only a
24x12 mode corner, so they are computed as real DFT matmuls (cos/sin
factors) instead of full FFTs. Device stage 1: enc conv + tanh + forward
partial DFT. Host: per-mode transform on the tiny [32,32,24,12] mode
tensor (float64). Device stage 2: inverse partial DFT + w0/dec convs.
x0 stays device-resident between the stages.
"""
import os
import numpy as np

B, CIN, COUT, OP = 32, 6, 4, 32
MX, MY, DEC = 12, 12, 6
H, W = 192, 192
NDEV = 8
BS = B // NDEV  # 4 samples per core


def _precompute(km_r, km_i):
    K = km_r.astype(np.float64) + 1j * km_i.astype(np.float64)  # [t,f,x,y]
    I = np.eye(OP)
    A6 = np.zeros((MX, MY, OP, OP), dtype=np.complex128)
    for x in range(MX):
        for y in range(MY):
            A6[x, y] = np.linalg.matrix_power(I + K[:, :, x, y], DEC) - I
    K00 = K[:, :, 0, 0]
    Kr, Ki = K00.real, K00.imag
    S = sum(np.linalg.matrix_power(I + Kr, j) for j in range(DEC))
    mats = dict(
        A6=A6,
        A0=np.linalg.matrix_power(I + Kr, DEC) - I,
        B0=-(Ki @ S),
        Pk=np.stack([np.linalg.matrix_power(
            I + (K[:, :, k, 0] + np.conj(K[:, :, MX - k, 0])) / 2, DEC) - I
            for k in range(1, MX)]),
        R=np.linalg.matrix_power(I + K00 / 2, DEC) - I,
    )
    ky = np.arange(MY)
    w = np.arange(W)
    h = np.arange(H)
    kx24 = np.concatenate([np.arange(MX), np.arange(H - MX, H)])
    kx25 = np.concatenate([np.arange(MX), [MX], np.arange(H - MX, H)])
    s = np.full(MY, 2.0 / (H * W)); s[0] = 1.0 / (H * W)
    f32 = lambda a: np.ascontiguousarray(a, dtype=np.float32)
    dft = dict(
        Cw=f32(np.cos(2 * np.pi * np.outer(w, ky) / W)),
        Sw=f32(np.sin(2 * np.pi * np.outer(w, ky) / W)),
        Ch24=f32(np.cos(2 * np.pi * np.outer(h, kx24) / H)),
        Sh24=f32(np.sin(2 * np.pi * np.outer(h, kx24) / H)),
        Che=f32(np.cos(2 * np.pi * np.outer(kx25, h) / H)),
        She=f32(np.sin(2 * np.pi * np.outer(kx25, h) / H)),
        A2=f32(s[:, None] * np.cos(2 * np.pi * np.outer(ky, w) / W)),
        B2=f32(s[:, None] * np.sin(2 * np.pi * np.outer(ky, w) / W)),
    )
    return mats, dft


def _mode_transform(G_lo, G_hi, m):
    """G_lo/G_hi: [N,OP,12,12] complex128 -> DF [N,OP,25,12] complex128."""
    n = G_lo.shape[0]
    A6 = m['A6'].copy()
    A6[:, 0] = 0.0  # ky=0 handled below
    DG_lo = np.einsum('ntxy,xytf->nfxy', G_lo, A6, optimize=True)
    DG_hi = np.einsum('ntxy,xytf->nfxy', G_hi, A6, optimize=True)
    ur, ui = G_lo[:, :, 0, 0].real, G_lo[:, :, 0, 0].imag
    DG_lo[:, :, 0, 0] = ur @ m['A0'] + ui @ m['B0']
    for k in range(1, MX):
        sv = G_lo[:, :, k, 0] + np.conj(G_hi[:, :, MX - k, 0])
        D = (sv @ m['Pk'][k - 1]) / 2
        DG_lo[:, :, k, 0] = D
        DG_hi[:, :, MX - k, 0] = np.conj(D)
    u = G_hi[:, :, 0, 0]
    Dh0 = u @ m['R']
    DG_hi[:, :, 0, 0] = Dh0
    DF = np.zeros((n, OP, 2 * MX + 1, MY), dtype=np.complex128)
    DF[:, :, :MX] = DG_lo
    DF[:, :, MX, 0] = np.conj(Dh0)  # leak at kx=12
    DF[:, :, MX + 1:] = DG_hi
    return DF


def _stage1(x, enc_w, enc_b, Cw, Sw, Ch24, Sh24):
    import jax.numpy as jnp
    x0 = jnp.tanh(jnp.einsum('bchw,oc->bohw', x, enc_w)
                  + enc_b[None, :, None, None])
    Tr = jnp.einsum('bthw,wy->bthy', x0, Cw)
    Ti = -jnp.einsum('bthw,wy->bthy', x0, Sw)
    Gr = (jnp.einsum('bthy,hx->btxy', Tr, Ch24)
          + jnp.einsum('bthy,hx->btxy', Ti, Sh24))
    Gi = (jnp.einsum('bthy,hx->btxy', Ti, Ch24)
          - jnp.einsum('bthy,hx->btxy', Tr, Sh24))
    return x0, Gr, Gi


def _stage2(x0, DFr, DFi, w0_w, w0_b, dec_w, dec_b, Che, She, A2, B2):
    import jax.numpy as jnp
    Er = (jnp.einsum('bfxy,xh->bfhy', DFr, Che)
          - jnp.einsum('bfxy,xh->bfhy', DFi, She))
    Ei = (jnp.einsum('bfxy,xh->bfhy', DFr, She)
          + jnp.einsum('bfxy,xh->bfhy', DFi, Che))
    d = (jnp.einsum('bfhy,yw->bfhw', Er, A2)
         - jnp.einsum('bfhy,yw->bfhw', Ei, B2))
    x_raw = (jnp.einsum('bchw,oc->bohw', x0, w0_w)
             + w0_b[None, :, None, None] + x0 + d)
    out = (jnp.einsum('bchw,oc->bohw', jnp.tanh(x_raw), dec_w)
           + dec_b[None, :, None, None])
    return out, x_raw.reshape(BS, OP, H * W)


_CACHE = {}


def kernel(x, enc_w, enc_b, dec_w, dec_b, w0_w, w0_b, km_r, km_i):
    os.environ.setdefault('NEURON_CC_FLAGS', '--auto-cast=none')
    import jax
    mats, dft = _precompute(np.asarray(km_r), np.asarray(km_i))
    if 'f1' not in _CACHE:
        devs = jax.devices()[:NDEV]
        _CACHE['f1'] = jax.pmap(_stage1, in_axes=(0,) + (None,) * 6,
                                devices=devs)
        _CACHE['f2'] = jax.pmap(_stage2, in_axes=(0, 0, 0) + (None,) * 8,
                                devices=devs)
    xs = np.ascontiguousarray(np.asarray(x, np.float32).reshape(
        NDEV, BS, CIN, H, W))
    f32 = lambda a: np.asarray(a, np.float32)
    x0, Gr, Gi = _CACHE['f1'](xs, f32(enc_w), f32(enc_b), dft['Cw'],
                              dft['Sw'], dft['Ch24'], dft['Sh24'])
    G = (np.asarray(Gr, np.float64)
         + 1j * np.asarray(Gi, np.float64)).reshape(B, OP, 2 * MX, MY)
    DF = _mode_transform(G[:, :, :MX], G[:, :, MX:], mats)
    DFr = np.ascontiguousarray(
        DF.real.reshape(NDEV, BS, OP, 2 * MX + 1, MY), np.float32)
    DFi = np.ascontiguousarray(
        DF.imag.reshape(NDEV, BS, OP, 2 * MX + 1, MY), np.float32)
    out, x_raw = _CACHE['f2'](x0, DFr, DFi, f32(w0_w), f32(w0_b),
                              f32(dec_w), f32(dec_b), dft['Che'],
                              dft['She'], dft['A2'], dft['B2'])
    out = np.asarray(out).reshape(B, COUT, H, W).astype(np.float32)
    x_raw = np.asarray(x_raw).reshape(B, OP, H * W).astype(np.float32)
    return out, x_raw
